# revision 10
# baseline (speedup 1.0000x reference)
"""MoE (16 experts, top-2) + shared SwiGLU expert — Trainium2 Bass kernel.

Strategy (8 NeuronCores, SPMD):
  - Router runs on host (tiny: 2048x1024x16). Tokens are grouped by expert.
  - Expert-parallel: core c owns experts {2c, 2c+1}; host gathers the tokens
    routed to each expert (padded to capacity C) and ships them transposed
    (features-on-partitions) so no on-device transposes are needed.
  - Shared expert is FF-sharded: core c computes a 352-wide slice of the
    2816-wide SwiGLU FF; host sums the 8 partial down-projections.
  - All matmul inputs are cast to bf16 on host (fp32 accumulation in PSUM).
  - Host combine: out = sum(partials).T * sigmoid(x@sgw.T) + scatter(expert).
"""

import os

import numpy as np
import ml_dtypes

import concourse.bass as bass
import concourse.mybir as mybir
import concourse.tile as tile
from concourse.bass_utils import run_bass_kernel_spmd

HIDDEN = 1024
MOE_FF = 512
SHARED_FF = 2816
NUM_EXPERTS = 16
TOP_K = 2
N_CORES = 8
NTOK = 2048
FF_SH = SHARED_FF // N_CORES  # 352
P = 128
KD = HIDDEN // P  # 8 contraction chunks over hidden
FF_CH = [(0, 128), (128, 128), (256, 96)]  # shared-FF shard chunking (352)
TT = 512  # token tile (PSUM free-dim limit)
NT = NTOK // TT

BF16 = ml_dtypes.bfloat16

_prog_cache: dict = {}


def _split_excess_waits(nc: bass.Bass) -> None:
    """This container's walrus accepts at most 1 sync-wait per instruction
    (2 on EventSemaphore), but Tile's tail barrier can emit more; split the
    excess onto preceding EventSemaphore instructions on the same engine."""
    for fn in nc.m.functions:
        for blk in fn.blocks:
            out = []
            for ins in blk.instructions:
                si = ins.sync_info
                cap = 2 if isinstance(ins, mybir.InstEventSemaphore) else 1
                if si is not None and len(si.on_wait) > cap:
                    waits = list(si.on_wait)
                    excess, keep = waits[:-cap], waits[-cap:]
                    for i in range(0, len(excess), 2):
                        ev = mybir.InstEventSemaphore(
                            name=nc.get_next_instruction_name(), ins=[], outs=[])
                        ev.engine = ins.engine
                        ev.sync_info = mybir.SyncInfo(
                            on_wait=excess[i:i + 2], on_update=[])
                        nc.register_instruction(ev)
                        out.append(ev)
                    si.on_wait = keep
                out.append(ins)
            blk.instructions[:] = out


def _build_program(C: int) -> bass.Bass:
    """Per-core program. C = per-expert token capacity (multiple of 32)."""
    nc = bass.Bass()
    dt = mybir.dt
    f = mybir.ActivationFunctionType

    xt_d = nc.dram_tensor("xt", [HIDDEN, NTOK], dt.bfloat16, kind="ExternalInput")
    xe_d = nc.dram_tensor("xe", [2, HIDDEN, C], dt.bfloat16, kind="ExternalInput")
    # wgu repacked on host: [e, pair j, hidden, 256] with cols = gate_j | up_j
    wgu_d = nc.dram_tensor("wgu", [2, 4, HIDDEN, 2 * P], dt.bfloat16, kind="ExternalInput")
    wd_d = nc.dram_tensor("wd", [2, MOE_FF, HIDDEN], dt.bfloat16, kind="ExternalInput")
    wg_d = nc.dram_tensor("wg_t", [HIDDEN, FF_SH], dt.bfloat16, kind="ExternalInput")
    wu_d = nc.dram_tensor("wu_t", [HIDDEN, FF_SH], dt.bfloat16, kind="ExternalInput")
    wdt_d = nc.dram_tensor("wdt", [FF_SH, HIDDEN], dt.bfloat16, kind="ExternalInput")
    ymoe_d = nc.dram_tensor("y_moe", [HIDDEN, 2 * C], dt.bfloat16, kind="ExternalOutput")
    ysh_d = nc.dram_tensor("y_sh", [HIDDEN, NTOK], dt.bfloat16, kind="ExternalOutput")

    with tile.TileContext(nc) as tc:
        with (
            tc.tile_pool(name="res", bufs=1) as res,
            tc.tile_pool(name="wmoe", bufs=2) as wmoe,
            tc.tile_pool(name="acts", bufs=3) as acts,
            tc.tile_pool(name="outs", bufs=2) as outs,
            tc.tile_pool(name="psum", bufs=2, space="PSUM") as psum,
        ):
            # ---- loads, in consumption order ----
            xe = []
            wgu = [[None] * 4, [None] * 4]
            for e in range(2):
                t = res.tile([P, KD, C], dt.bfloat16, tag=f"xe{e}", name=f"xe{e}")
                nc.sync.dma_start(t[:], xe_d[e].rearrange("(k p) t -> p k t", p=P))
                xe.append(t)
                for j in range(4):
                    wt = wmoe.tile([P, KD, 2 * P], dt.bfloat16, tag=f"wgu{e}_{j}", name=f"wgu{e}_{j}")
                    nc.sync.dma_start(wt[:], wgu_d[e, j].rearrange("(k p) f -> p k f", p=P))
                    wgu[e][j] = wt
            wdl = []
            for e in range(2):
                dl = wmoe.tile([P, MOE_FF // P, HIDDEN], dt.bfloat16, tag=f"wd{e}", name=f"wd{e}")
                nc.sync.dma_start(dl[:], wd_d[e].rearrange("(j p) d -> p j d", p=P))
                wdl.append(dl)
            xt = res.tile([P, KD, NTOK], dt.bfloat16, tag="xt", name="xt")
            nc.sync.dma_start(xt[:], xt_d.rearrange("(k p) t -> p k t", p=P))
            wg = res.tile([P, KD, FF_SH], dt.bfloat16, tag="wg", name="wg")
            nc.sync.dma_start(wg[:], wg_d.rearrange("(k p) t -> p k t", p=P))
            wu = res.tile([P, KD, FF_SH], dt.bfloat16, tag="wu", name="wu")
            nc.sync.dma_start(wu[:], wu_d.rearrange("(k p) t -> p k t", p=P))
            wdt = []
            for j, (off, sz) in enumerate(FF_CH):
                t = res.tile([P, HIDDEN], dt.bfloat16, tag=f"wdt{j}", name=f"wdt{j}")
                nc.sync.dma_start(t[:sz, :], wdt_d[off:off + sz, :])
                wdt.append(t)

            ymoe_r = ymoe_d.rearrange("(m p) t -> p m t", p=P)
            ysh_r = ysh_d.rearrange("(m p) t -> p m t", p=P)

            # ---- MoE experts (2 per core) ----
            for e in range(2):
                hs = []
                for j in range(4):  # gate/up feature pairs of 2*MOE_FF
                    pg = psum.tile([P, TT], dt.float32, tag="pg", name=f"pg{e}_{j}")
                    pu = psum.tile([P, TT], dt.float32, tag="pu", name=f"pu{e}_{j}")
                    for k in range(KD):
                        nc.tensor.matmul(
                            pg[:, :C], wgu[e][j][:, k, 0:P], xe[e][:, k, :],
                            start=(k == 0), stop=(k == KD - 1),
                        )
                    for k in range(KD):
                        nc.tensor.matmul(
                            pu[:, :C], wgu[e][j][:, k, P:2 * P], xe[e][:, k, :],
                            start=(k == 0), stop=(k == KD - 1),
                        )
                    sg = acts.tile([P, TT], dt.bfloat16, tag="sg", name=f"sg{e}_{j}")
                    nc.scalar.activation(sg[:, :C], pg[:, :C], f.Silu)
                    h = acts.tile([P, TT], dt.bfloat16, tag=f"h{j}", name=f"h{e}_{j}")
                    nc.vector.tensor_tensor(h[:, :C], sg[:, :C], pu[:, :C], mybir.AluOpType.mult)
                    hs.append(h)
                yo = outs.tile([P, KD, C], dt.bfloat16, tag="ymoe", name=f"ymoe{e}")
                for m in range(KD):  # output feature chunks of HIDDEN
                    pd = psum.tile([P, TT], dt.float32, tag="pd", bufs=3, name=f"pd{e}_{m}")
                    for j in range(4):
                        nc.tensor.matmul(
                            pd[:, :C], wdl[e][:, j, m * P:(m + 1) * P], hs[j][:, :C],
                            start=(j == 0), stop=(j == 3),
                        )
                    if m % 2 == 0:
                        nc.vector.tensor_copy(yo[:, m, :], pd[:, :C])
                    else:
                        nc.scalar.activation(yo[:, m, :], pd[:, :C], f.Copy)
                nc.gpsimd.dma_start(ymoe_r[:, :, e * C:(e + 1) * C], yo[:])

            # ---- shared expert shard ----
            for t_i in range(NT):
                tsl = slice(t_i * TT, (t_i + 1) * TT)
                hsh = []
                for j, (off, sz) in enumerate(FF_CH):
                    pg = psum.tile([P, TT], dt.float32, tag="pg", name=f"spg{t_i}_{j}")
                    pu = psum.tile([P, TT], dt.float32, tag="pu", name=f"spu{t_i}_{j}")
                    for k in range(KD):
                        nc.tensor.matmul(
                            pg[:sz, :], wg[:, k, off:off + sz], xt[:, k, tsl],
                            start=(k == 0), stop=(k == KD - 1),
                        )
                    for k in range(KD):
                        nc.tensor.matmul(
                            pu[:sz, :], wu[:, k, off:off + sz], xt[:, k, tsl],
                            start=(k == 0), stop=(k == KD - 1),
                        )
                    sg = acts.tile([P, TT], dt.bfloat16, tag="sg", name=f"ssg{t_i}_{j}")
                    nc.scalar.activation(sg[:sz, :], pg[:sz, :], f.Silu)
                    h = acts.tile([P, TT], dt.bfloat16, tag=f"h{j}", name=f"sh{t_i}_{j}")
                    nc.vector.tensor_tensor(h[:sz, :], sg[:sz, :], pu[:sz, :], mybir.AluOpType.mult)
                    hsh.append(h)
                ys = outs.tile([P, KD, TT], dt.bfloat16, tag="ysh", name=f"ysh{t_i}")
                for m in range(KD):
                    pd = psum.tile([P, TT], dt.float32, tag="pd", bufs=3, name=f"spd{t_i}_{m}")
                    for j, (off, sz) in enumerate(FF_CH):
                        nc.tensor.matmul(
                            pd[:], wdt[j][:sz, m * P:(m + 1) * P], hsh[j][:sz, :],
                            start=(j == 0), stop=(j == 2),
                        )
                    if m % 2 == 0:
                        nc.vector.tensor_copy(ys[:, m, :], pd[:])
                    else:
                        nc.scalar.activation(ys[:, m, :], pd[:], f.Copy)
                    if m == 3:
                        nc.gpsimd.dma_start(ysh_r[:, 0:4, tsl], ys[:, 0:4, :])
                nc.gpsimd.dma_start(ysh_r[:, 4:8, tsl], ys[:, 4:8, :])
    _split_excess_waits(nc)
    return nc


def _route(x: np.ndarray, gate_w: np.ndarray):
    logits = x @ gate_w.T
    logits = logits.astype(np.float32)
    m = logits.max(axis=-1, keepdims=True)
    p = np.exp(logits - m)
    p /= p.sum(axis=-1, keepdims=True)
    sel = np.argsort(-p, axis=-1, kind="stable")[:, :TOP_K]
    rw = np.take_along_axis(p, sel, axis=-1)
    rw = rw / rw.sum(axis=-1, keepdims=True)
    idxs, wts = [], []
    for e in range(NUM_EXPERTS):
        mask = (sel == e).any(axis=-1)
        idx = np.nonzero(mask)[0]
        w = rw[idx][sel[idx] == e]
        idxs.append(idx)
        wts.append(w.astype(np.float32))
    return idxs, wts


def kernel(layer_input, gate_w, w_gate_up, w_down,
           shared_w_gate, shared_w_up, shared_w_down, shared_gate_w):
    B, S, D = layer_input.shape
    x = np.ascontiguousarray(np.asarray(layer_input, dtype=np.float32).reshape(-1, D))

    idxs, wts = _route(x, np.asarray(gate_w, dtype=np.float32))
    cmax = max(len(i) for i in idxs)
    C = max(32, ((cmax + 31) // 32) * 32)

    key = C
    if key not in _prog_cache:
        _prog_cache[key] = _build_program(C)
    nc = _prog_cache[key]

    xt = np.ascontiguousarray(x.T).astype(BF16)
    wgu_f = np.asarray(w_gate_up, dtype=np.float32)
    # repack into [E, pair j, hidden, 256] with cols = gate_j | up_j
    wgu_pk = np.empty((NUM_EXPERTS, 4, HIDDEN, 2 * P), dtype=BF16)
    for j in range(4):
        wgu_pk[:, j, :, :P] = wgu_f[:, :, j * P:(j + 1) * P]
        wgu_pk[:, j, :, P:] = wgu_f[:, :, MOE_FF + j * P:MOE_FF + (j + 1) * P]
    wd_all = np.asarray(w_down, dtype=np.float32).astype(BF16)
    wg_t_all = np.ascontiguousarray(np.asarray(shared_w_gate, np.float32).T).astype(BF16)
    wu_t_all = np.ascontiguousarray(np.asarray(shared_w_up, np.float32).T).astype(BF16)
    wdt_all = np.ascontiguousarray(np.asarray(shared_w_down, np.float32).T).astype(BF16)

    in_maps = []
    for c in range(N_CORES):
        xe = np.zeros((2, C, HIDDEN), dtype=BF16)
        for s_i, e in enumerate((2 * c, 2 * c + 1)):
            cnt = len(idxs[e])
            xe[s_i, :cnt] = x[idxs[e]].astype(BF16)
        fsl = slice(c * FF_SH, (c + 1) * FF_SH)
        in_maps.append({
            "xt": xt,
            "xe": np.ascontiguousarray(xe.transpose(0, 2, 1)),
            "wgu": np.ascontiguousarray(wgu_pk[2 * c:2 * c + 2]),
            "wd": np.ascontiguousarray(wd_all[2 * c:2 * c + 2]),
            "wg_t": np.ascontiguousarray(wg_t_all[:, fsl]),
            "wu_t": np.ascontiguousarray(wu_t_all[:, fsl]),
            "wdt": np.ascontiguousarray(wdt_all[fsl, :]),
        })

    trace = bool(int(os.environ.get("BASS_MOE_TRACE", "0")))
    res = run_bass_kernel_spmd(
        nc, in_maps, core_ids=list(range(N_CORES)),
        trace=trace, trace_cores=list(range(N_CORES)) if trace else None,
    )
    kernel.last_results = res

    shared = np.zeros((HIDDEN, NTOK), dtype=np.float32)
    for c in range(N_CORES):
        shared += np.asarray(res.results[c]["y_sh"]).astype(np.float32)
    sig = 1.0 / (1.0 + np.exp(-(x @ np.asarray(shared_gate_w, np.float32).T)))
    out = shared.T * sig
    for e in range(NUM_EXPERTS):
        c, s_i = e // 2, e % 2
        cnt = len(idxs[e])
        if cnt == 0:
            continue
        ye = np.asarray(res.results[c]["y_moe"]).astype(np.float32)[:, s_i * C:s_i * C + cnt]
        out[idxs[e]] += wts[e][:, None] * ye.T
    return out.reshape(B, S, D).astype(np.float32)


# revision 11
# speedup vs baseline: 1.0488x; 1.0488x over previous
"""MoE (16 experts, top-2) + shared SwiGLU expert — Trainium2 Bass kernel.

Strategy (8 NeuronCores, SPMD):
  - Router runs on host (tiny: 2048x1024x16). Tokens are grouped by expert.
  - Expert-parallel: core c owns experts {2c, 2c+1}; host gathers the tokens
    routed to each expert (padded to capacity C) and ships them transposed
    (features-on-partitions) so no on-device transposes are needed.
  - Shared expert is FF-sharded: core c computes a 352-wide slice of the
    2816-wide SwiGLU FF; host sums the 8 partial down-projections.
  - All matmul inputs are cast to bf16 on host (fp32 accumulation in PSUM).
  - All device tensors are host-pre-permuted into SBUF-native [128, ...]
    layouts so every DMA is one contiguous block per partition row.
  - Host combine: out = sum(partials).T * sigmoid(x@sgw.T) + scatter(expert).
"""

import os

import numpy as np
import ml_dtypes

import concourse.bass as bass
import concourse.mybir as mybir
import concourse.tile as tile
from concourse.bass_utils import run_bass_kernel_spmd

HIDDEN = 1024
MOE_FF = 512
SHARED_FF = 2816
NUM_EXPERTS = 16
TOP_K = 2
N_CORES = 8
NTOK = 2048
FF_SH = SHARED_FF // N_CORES  # 352
P = 128
KD = HIDDEN // P  # 8 contraction chunks over hidden
FF_CH = [(0, 128), (128, 128), (256, 96)]  # shared-FF shard chunking (352)
TT = 512  # token tile (PSUM free-dim limit)
NT = NTOK // TT

BF16 = ml_dtypes.bfloat16

_prog_cache: dict = {}


def _split_excess_waits(nc: bass.Bass) -> None:
    """This container's walrus accepts at most 1 sync-wait per instruction
    (2 on EventSemaphore), but Tile's tail barrier can emit more; split the
    excess onto preceding EventSemaphore instructions on the same engine."""
    for fn in nc.m.functions:
        for blk in fn.blocks:
            out = []
            for ins in blk.instructions:
                si = ins.sync_info
                cap = 2 if isinstance(ins, mybir.InstEventSemaphore) else 1
                if si is not None and len(si.on_wait) > cap:
                    waits = list(si.on_wait)
                    excess, keep = waits[:-cap], waits[-cap:]
                    for i in range(0, len(excess), 2):
                        ev = mybir.InstEventSemaphore(
                            name=nc.get_next_instruction_name(), ins=[], outs=[])
                        ev.engine = ins.engine
                        ev.sync_info = mybir.SyncInfo(
                            on_wait=excess[i:i + 2], on_update=[])
                        nc.register_instruction(ev)
                        out.append(ev)
                    si.on_wait = keep
                out.append(ins)
            blk.instructions[:] = out


def _pk(a: np.ndarray) -> np.ndarray:
    """[n*128, cols] -> [128, n*cols] partition-major contiguous pack."""
    n = a.shape[0] // P
    return np.ascontiguousarray(
        a.reshape(n, P, a.shape[1]).transpose(1, 0, 2).reshape(P, -1)).astype(BF16)


def _build_program(C: int) -> bass.Bass:
    """Per-core program. C = per-expert token capacity (multiple of 32)."""
    nc = bass.Bass()
    dt = mybir.dt
    f = mybir.ActivationFunctionType

    xt_d = nc.dram_tensor("xt", [P, KD * NTOK], dt.bfloat16, kind="ExternalInput")
    xe_d = nc.dram_tensor("xe", [2, P, KD * C], dt.bfloat16, kind="ExternalInput")
    wgu_d = nc.dram_tensor("wgu", [2, 4, P, KD * 2 * P], dt.bfloat16, kind="ExternalInput")
    wd_d = nc.dram_tensor("wd", [2, P, (MOE_FF // P) * HIDDEN], dt.bfloat16, kind="ExternalInput")
    wg_d = nc.dram_tensor("wg_t", [P, KD * FF_SH], dt.bfloat16, kind="ExternalInput")
    wu_d = nc.dram_tensor("wu_t", [P, KD * FF_SH], dt.bfloat16, kind="ExternalInput")
    wdt_d = nc.dram_tensor("wdt", [FF_SH, HIDDEN], dt.bfloat16, kind="ExternalInput")
    # outputs in device-native layouts; host inverse-permutes
    ymoe_d = nc.dram_tensor("y_moe", [2, P, KD * C], dt.bfloat16, kind="ExternalOutput")
    ysh_d = nc.dram_tensor("y_sh", [NT, P, KD * TT], dt.bfloat16, kind="ExternalOutput")

    with tile.TileContext(nc) as tc:
        with (
            tc.tile_pool(name="res", bufs=1) as res,
            tc.tile_pool(name="wmoe", bufs=2) as wmoe,
            tc.tile_pool(name="acts", bufs=3) as acts,
            tc.tile_pool(name="outs", bufs=2) as outs,
            tc.tile_pool(name="psum", bufs=2, space="PSUM") as psum,
        ):
            # ---- loads, in consumption order; all DMAs contiguous ----
            xe = []
            wgu = [[None] * 4, [None] * 4]
            for e in range(2):
                t = res.tile([P, KD, C], dt.bfloat16, tag=f"xe{e}", name=f"xe{e}")
                nc.sync.dma_start(t[:], xe_d[e].rearrange("p (k t) -> p k t", k=KD))
                xe.append(t)
                for j in range(4):
                    wt = wmoe.tile([P, KD, 2 * P], dt.bfloat16, tag=f"wgu{e}_{j}", name=f"wgu{e}_{j}")
                    nc.sync.dma_start(wt[:], wgu_d[e, j].rearrange("p (k t) -> p k t", k=KD))
                    wgu[e][j] = wt
            wdl = []
            for e in range(2):
                dl = wmoe.tile([P, MOE_FF // P, HIDDEN], dt.bfloat16, tag=f"wd{e}", name=f"wd{e}")
                nc.sync.dma_start(dl[:], wd_d[e].rearrange("p (k t) -> p k t", k=MOE_FF // P))
                wdl.append(dl)
            xt = res.tile([P, KD, NTOK], dt.bfloat16, tag="xt", name="xt")
            nc.sync.dma_start(xt[:], xt_d.rearrange("p (k t) -> p k t", k=KD))
            wg = res.tile([P, KD, FF_SH], dt.bfloat16, tag="wg", name="wg")
            nc.sync.dma_start(wg[:], wg_d.rearrange("p (k t) -> p k t", k=KD))
            wu = res.tile([P, KD, FF_SH], dt.bfloat16, tag="wu", name="wu")
            nc.sync.dma_start(wu[:], wu_d.rearrange("p (k t) -> p k t", k=KD))
            wdt = []
            for j, (off, sz) in enumerate(FF_CH):
                t = res.tile([P, HIDDEN], dt.bfloat16, tag=f"wdt{j}", name=f"wdt{j}")
                nc.sync.dma_start(t[:sz, :], wdt_d[off:off + sz, :])
                wdt.append(t)

            # ---- MoE experts (2 per core) ----
            for e in range(2):
                hs = []
                for j in range(4):  # gate/up feature pairs of 2*MOE_FF
                    pg = psum.tile([P, TT], dt.float32, tag="pg", name=f"pg{e}_{j}")
                    pu = psum.tile([P, TT], dt.float32, tag="pu", name=f"pu{e}_{j}")
                    for k in range(KD):
                        nc.tensor.matmul(
                            pg[:, :C], wgu[e][j][:, k, 0:P], xe[e][:, k, :],
                            start=(k == 0), stop=(k == KD - 1),
                        )
                    for k in range(KD):
                        nc.tensor.matmul(
                            pu[:, :C], wgu[e][j][:, k, P:2 * P], xe[e][:, k, :],
                            start=(k == 0), stop=(k == KD - 1),
                        )
                    sg = acts.tile([P, TT], dt.bfloat16, tag="sg", name=f"sg{e}_{j}")
                    nc.scalar.activation(sg[:, :C], pg[:, :C], f.Silu)
                    h = acts.tile([P, TT], dt.bfloat16, tag=f"h{j}", name=f"h{e}_{j}")
                    nc.vector.tensor_tensor(h[:, :C], sg[:, :C], pu[:, :C], mybir.AluOpType.mult)
                    hs.append(h)
                yo = outs.tile([P, KD, C], dt.bfloat16, tag="ymoe", name=f"ymoe{e}")
                for m in range(KD):  # output feature chunks of HIDDEN
                    pd = psum.tile([P, TT], dt.float32, tag="pd", bufs=3, name=f"pd{e}_{m}")
                    for j in range(4):
                        nc.tensor.matmul(
                            pd[:, :C], wdl[e][:, j, m * P:(m + 1) * P], hs[j][:, :C],
                            start=(j == 0), stop=(j == 3),
                        )
                    if m % 2 == 0:
                        nc.vector.tensor_copy(yo[:, m, :], pd[:, :C])
                    else:
                        nc.scalar.activation(yo[:, m, :], pd[:, :C], f.Copy)
                nc.gpsimd.dma_start(ymoe_d[e].rearrange("p (k t) -> p k t", k=KD), yo[:])

            # ---- shared expert shard ----
            for t_i in range(NT):
                tsl = slice(t_i * TT, (t_i + 1) * TT)
                ysh_t = ysh_d[t_i].rearrange("p (k t) -> p k t", k=KD)
                hsh = []
                for j, (off, sz) in enumerate(FF_CH):
                    pg = psum.tile([P, TT], dt.float32, tag="pg", name=f"spg{t_i}_{j}")
                    pu = psum.tile([P, TT], dt.float32, tag="pu", name=f"spu{t_i}_{j}")
                    for k in range(KD):
                        nc.tensor.matmul(
                            pg[:sz, :], wg[:, k, off:off + sz], xt[:, k, tsl],
                            start=(k == 0), stop=(k == KD - 1),
                        )
                    for k in range(KD):
                        nc.tensor.matmul(
                            pu[:sz, :], wu[:, k, off:off + sz], xt[:, k, tsl],
                            start=(k == 0), stop=(k == KD - 1),
                        )
                    sg = acts.tile([P, TT], dt.bfloat16, tag="sg", name=f"ssg{t_i}_{j}")
                    nc.scalar.activation(sg[:sz, :], pg[:sz, :], f.Silu)
                    h = acts.tile([P, TT], dt.bfloat16, tag=f"h{j}", name=f"sh{t_i}_{j}")
                    nc.vector.tensor_tensor(h[:sz, :], sg[:sz, :], pu[:sz, :], mybir.AluOpType.mult)
                    hsh.append(h)
                ys = outs.tile([P, KD, TT], dt.bfloat16, tag="ysh", name=f"ysh{t_i}")
                for m in range(KD):
                    pd = psum.tile([P, TT], dt.float32, tag="pd", bufs=3, name=f"spd{t_i}_{m}")
                    for j, (off, sz) in enumerate(FF_CH):
                        nc.tensor.matmul(
                            pd[:], wdt[j][:sz, m * P:(m + 1) * P], hsh[j][:sz, :],
                            start=(j == 0), stop=(j == 2),
                        )
                    if m % 2 == 0:
                        nc.vector.tensor_copy(ys[:, m, :], pd[:])
                    else:
                        nc.scalar.activation(ys[:, m, :], pd[:], f.Copy)
                    if m == 3:
                        nc.gpsimd.dma_start(ysh_t[:, 0:4, :], ys[:, 0:4, :])
                nc.gpsimd.dma_start(ysh_t[:, 4:8, :], ys[:, 4:8, :])
    _split_excess_waits(nc)
    return nc


def _route(x: np.ndarray, gate_w: np.ndarray):
    logits = x @ gate_w.T
    logits = logits.astype(np.float32)
    m = logits.max(axis=-1, keepdims=True)
    p = np.exp(logits - m)
    p /= p.sum(axis=-1, keepdims=True)
    sel = np.argsort(-p, axis=-1, kind="stable")[:, :TOP_K]
    rw = np.take_along_axis(p, sel, axis=-1)
    rw = rw / rw.sum(axis=-1, keepdims=True)
    idxs, wts = [], []
    for e in range(NUM_EXPERTS):
        mask = (sel == e).any(axis=-1)
        idx = np.nonzero(mask)[0]
        w = rw[idx][sel[idx] == e]
        idxs.append(idx)
        wts.append(w.astype(np.float32))
    return idxs, wts


def kernel(layer_input, gate_w, w_gate_up, w_down,
           shared_w_gate, shared_w_up, shared_w_down, shared_gate_w):
    B, S, D = layer_input.shape
    x = np.ascontiguousarray(np.asarray(layer_input, dtype=np.float32).reshape(-1, D))

    idxs, wts = _route(x, np.asarray(gate_w, dtype=np.float32))
    cmax = max(len(i) for i in idxs)
    C = max(32, ((cmax + 31) // 32) * 32)

    key = C
    if key not in _prog_cache:
        _prog_cache[key] = _build_program(C)
    nc = _prog_cache[key]

    xt_pk = _pk(x.T)  # [128, 8*2048]
    wgu_f = np.asarray(w_gate_up, dtype=np.float32)
    wd_f = np.asarray(w_down, dtype=np.float32)
    wg_t = np.asarray(shared_w_gate, np.float32).T  # [1024, 2816]
    wu_t = np.asarray(shared_w_up, np.float32).T
    wdt_t = np.ascontiguousarray(np.asarray(shared_w_down, np.float32).T).astype(BF16)

    in_maps = []
    for c in range(N_CORES):
        e0, e1 = 2 * c, 2 * c + 1
        xe = np.zeros((2, P, KD * C), dtype=BF16)
        wgu = np.zeros((2, 4, P, KD * 2 * P), dtype=BF16)
        wd = np.zeros((2, P, (MOE_FF // P) * HIDDEN), dtype=BF16)
        for s_i, e in enumerate((e0, e1)):
            cnt = len(idxs[e])
            xfull = np.zeros((C, HIDDEN), dtype=np.float32)
            xfull[:cnt] = x[idxs[e]]
            xe[s_i] = _pk(np.ascontiguousarray(xfull.T))
            for j in range(4):
                blk = np.concatenate(
                    [wgu_f[e][:, j * P:(j + 1) * P],
                     wgu_f[e][:, MOE_FF + j * P:MOE_FF + (j + 1) * P]], axis=1)
                wgu[s_i, j] = _pk(blk)
            wd[s_i] = _pk(wd_f[e])
        fsl = slice(c * FF_SH, (c + 1) * FF_SH)
        in_maps.append({
            "xt": xt_pk,
            "xe": xe,
            "wgu": wgu,
            "wd": wd,
            "wg_t": _pk(np.ascontiguousarray(wg_t[:, fsl])),
            "wu_t": _pk(np.ascontiguousarray(wu_t[:, fsl])),
            "wdt": wdt_t[fsl, :],
        })

    trace = bool(int(os.environ.get("BASS_MOE_TRACE", "0")))
    res = run_bass_kernel_spmd(
        nc, in_maps, core_ids=list(range(N_CORES)),
        trace=trace, trace_cores=list(range(N_CORES)) if trace else None,
    )
    kernel.last_results = res

    # unshard: y_sh [NT, 128, KD*TT] -> [1024, 2048]
    shared = np.zeros((HIDDEN, NTOK), dtype=np.float32)
    for c in range(N_CORES):
        ysh = np.asarray(res.results[c]["y_sh"]).astype(np.float32)
        ysh = ysh.reshape(NT, P, KD, TT).transpose(2, 1, 0, 3).reshape(HIDDEN, NTOK)
        shared += ysh
    sig = 1.0 / (1.0 + np.exp(-(x @ np.asarray(shared_gate_w, np.float32).T)))
    out = shared.T * sig
    for e in range(NUM_EXPERTS):
        c, s_i = e // 2, e % 2
        cnt = len(idxs[e])
        if cnt == 0:
            continue
        ym = np.asarray(res.results[c]["y_moe"][s_i]).astype(np.float32)
        ym = ym.reshape(P, KD, C).transpose(1, 0, 2).reshape(HIDDEN, C)
        out[idxs[e]] += wts[e][:, None] * ym[:, :cnt].T
    return out.reshape(B, S, D).astype(np.float32)


# revision 19
# speedup vs baseline: 1.1665x; 1.1122x over previous
"""MoE (16 experts, top-2) + shared SwiGLU expert — Trainium2 Bass kernel.

Strategy (8 NeuronCores, SPMD):
  - Router runs on host (tiny: 2048x1024x16). Tokens are grouped by expert.
  - Expert-parallel: core c owns experts {2c, 2c+1}; host gathers the tokens
    routed to each expert (padded to capacity C) and ships them transposed
    (features-on-partitions) so no on-device transposes are needed.
  - Shared expert is FF-sharded: core c computes a 352-wide slice of the
    2816-wide SwiGLU FF; host sums the 8 partial down-projections.
  - All matmul inputs are cast to bf16 on host (fp32 accumulation in PSUM).
  - All device tensors are host-pre-permuted into SBUF-native [128, ...]
    layouts so every DMA is one contiguous block per partition row.
  - Host combine: out = sum(partials).T * sigmoid(x@sgw.T) + scatter(expert).
"""

import os

import numpy as np
import ml_dtypes

import concourse.bass as bass
import concourse.mybir as mybir
import concourse.tile as tile
from concourse.tile import add_dep_helper
from concourse.bass_utils import run_bass_kernel_spmd

HIDDEN = 1024
MOE_FF = 512
SHARED_FF = 2816
NUM_EXPERTS = 16
TOP_K = 2
N_CORES = 8
NTOK = 2048
FF_SH = SHARED_FF // N_CORES  # 352
P = 128
KD = HIDDEN // P  # 8 contraction chunks over hidden
FF_CH = [(0, 128), (128, 128), (256, 96)]  # shared-FF shard chunking (352)
TT = 512  # token tile (PSUM free-dim limit)
NT = NTOK // TT

BF16 = ml_dtypes.bfloat16

_prog_cache: dict = {}


def _split_excess_waits(nc: bass.Bass) -> None:
    """This container's walrus accepts at most 1 sync-wait per instruction
    (2 on EventSemaphore), but Tile's tail barrier can emit more; split the
    excess onto preceding EventSemaphore instructions on the same engine."""
    for fn in nc.m.functions:
        for blk in fn.blocks:
            out = []
            for ins in blk.instructions:
                si = ins.sync_info
                cap = 2 if isinstance(ins, mybir.InstEventSemaphore) else 1
                if si is not None and len(si.on_wait) > cap:
                    waits = list(si.on_wait)
                    excess, keep = waits[:-cap], waits[-cap:]
                    for i in range(0, len(excess), 2):
                        ev = mybir.InstEventSemaphore(
                            name=nc.get_next_instruction_name(), ins=[], outs=[])
                        ev.engine = ins.engine
                        ev.sync_info = mybir.SyncInfo(
                            on_wait=excess[i:i + 2], on_update=[])
                        nc.register_instruction(ev)
                        out.append(ev)
                    si.on_wait = keep
                out.append(ins)
            blk.instructions[:] = out


def _pk(a: np.ndarray) -> np.ndarray:
    """[n*128, cols] -> [128, n*cols] partition-major contiguous pack."""
    n = a.shape[0] // P
    return np.ascontiguousarray(
        a.reshape(n, P, a.shape[1]).transpose(1, 0, 2).reshape(P, -1)).astype(BF16)


def _build_program(C: int) -> bass.Bass:
    """Per-core program. C = per-expert token capacity (multiple of 32)."""
    nc = bass.Bass()
    dt = mybir.dt
    f = mybir.ActivationFunctionType

    xt_d = nc.dram_tensor("xt", [2, P, KD * (NTOK // 2)], dt.bfloat16, kind="ExternalInput")
    xe_d = nc.dram_tensor("xe", [2, P, KD * C], dt.bfloat16, kind="ExternalInput")
    wgu_d = nc.dram_tensor("wgu", [2, 4, P, KD * 2 * P], dt.bfloat16, kind="ExternalInput")
    wd_d = nc.dram_tensor("wd", [2, P, (MOE_FF // P) * HIDDEN], dt.bfloat16, kind="ExternalInput")
    wg_d = nc.dram_tensor("wg_t", [P, KD * FF_SH], dt.bfloat16, kind="ExternalInput")
    wu_d = nc.dram_tensor("wu_t", [P, KD * FF_SH], dt.bfloat16, kind="ExternalInput")
    wdt_d = nc.dram_tensor("wdt", [FF_SH, HIDDEN], dt.bfloat16, kind="ExternalInput")
    # outputs in device-native layouts; host inverse-permutes
    ymoe_d = nc.dram_tensor("y_moe", [2, P, KD * C], dt.bfloat16, kind="ExternalOutput")
    ysh_d = nc.dram_tensor("y_sh", [NT, P, KD * TT], dt.bfloat16, kind="ExternalOutput")

    with tile.TileContext(nc) as tc:
        with (
            tc.tile_pool(name="res", bufs=1) as res,
            tc.tile_pool(name="wmoe", bufs=2) as wmoe,
            tc.tile_pool(name="acts", bufs=3) as acts,
            tc.tile_pool(name="outs", bufs=2) as outs,
            tc.tile_pool(name="psum", bufs=2, space="PSUM") as psum,
        ):
            # ---- loads; staged so early phases get full HBM bandwidth ----
            # stage 0 (immediate): expert-0 inputs
            xe = [None, None]
            wgu = [[None] * 4, [None] * 4]
            wdl = [None, None]
            stage_dmas = [[], [], [], []]

            def load_expert(e):
                t = res.tile([P, KD, C], dt.bfloat16, tag=f"xe{e}", name=f"xe{e}")
                d = nc.sync.dma_start(t[:], xe_d[e].rearrange("p (k t) -> p k t", k=KD))
                xe[e] = t
                dmas = [d]
                for j in range(4):
                    wt = wmoe.tile([P, KD, 2 * P], dt.bfloat16, tag=f"wgu{e}_{j}", name=f"wgu{e}_{j}")
                    dmas.append(nc.sync.dma_start(
                        wt[:], wgu_d[e, j].rearrange("p (k t) -> p k t", k=KD)))
                    wgu[e][j] = wt
                dl = wmoe.tile([P, MOE_FF // P, HIDDEN], dt.bfloat16, tag=f"wd{e}", name=f"wd{e}")
                dmas.append(nc.sync.dma_start(
                    dl[:], wd_d[e].rearrange("p (k t) -> p k t", k=MOE_FF // P)))
                wdl[e] = dl
                return dmas

            stage_dmas[0] = load_expert(0)
            # stage 1: expert-1 inputs
            stage_dmas[1] = load_expert(1)
            # stage 2: shared-expert first half
            xt = []
            for h in range(2):
                t = res.tile([P, KD, NTOK // 2], dt.bfloat16, tag=f"xt{h}", name=f"xt{h}")
                xt.append(t)
            stage_dmas[2].append(nc.sync.dma_start(
                xt[0][:], xt_d[0].rearrange("p (k t) -> p k t", k=KD)))
            wg = res.tile([P, KD, FF_SH], dt.bfloat16, tag="wg", name="wg")
            stage_dmas[2].append(nc.sync.dma_start(
                wg[:], wg_d.rearrange("p (k t) -> p k t", k=KD)))
            wu = res.tile([P, KD, FF_SH], dt.bfloat16, tag="wu", name="wu")
            stage_dmas[2].append(nc.sync.dma_start(
                wu[:], wu_d.rearrange("p (k t) -> p k t", k=KD)))
            # stage 3: shared down weights + second half tokens
            wdt = []
            for j, (off, sz) in enumerate(FF_CH):
                t = res.tile([P, HIDDEN], dt.bfloat16, tag=f"wdt{j}", name=f"wdt{j}")
                stage_dmas[3].append(nc.sync.dma_start(t[:sz, :], wdt_d[off:off + sz, :]))
                wdt.append(t)
            stage_dmas[3].append(nc.sync.dma_start(
                xt[1][:], xt_d[1].rearrange("p (k t) -> p k t", k=KD)))
            stage_gate = [None, None, None]  # first MM of e0, e1, shared

            # ---- MoE experts (2 per core) ----
            for e in range(2):
                hs = []
                for j in range(4):  # gate/up feature pairs of 2*MOE_FF
                    pg = psum.tile([P, TT], dt.float32, tag="pg", name=f"pg{e}_{j}")
                    pu = psum.tile([P, TT], dt.float32, tag="pu", name=f"pu{e}_{j}")
                    for k in range(KD):
                        mm = nc.tensor.matmul(
                            pg[:, :C], wgu[e][j][:, k, 0:P], xe[e][:, k, :],
                            start=(k == 0), stop=(k == KD - 1),
                        )
                        if j == 0 and k == 0:
                            stage_gate[e] = mm
                    for k in range(KD):
                        nc.tensor.matmul(
                            pu[:, :C], wgu[e][j][:, k, P:2 * P], xe[e][:, k, :],
                            start=(k == 0), stop=(k == KD - 1),
                        )
                    sg = acts.tile([P, TT], dt.bfloat16, tag="sg", name=f"sg{e}_{j}")
                    nc.scalar.activation(sg[:, :C], pg[:, :C], f.Silu)
                    h = acts.tile([P, TT], dt.bfloat16, tag=f"h{j}", name=f"h{e}_{j}")
                    nc.vector.tensor_tensor(h[:, :C], sg[:, :C], pu[:, :C], mybir.AluOpType.mult)
                    hs.append(h)
                yo = outs.tile([P, KD, C], dt.bfloat16, tag="ymoe", name=f"ymoe{e}")
                for m in range(KD):  # output feature chunks of HIDDEN
                    pd = psum.tile([P, TT], dt.float32, tag="pd", bufs=3, name=f"pd{e}_{m}")
                    for j in range(4):
                        nc.tensor.matmul(
                            pd[:, :C], wdl[e][:, j, m * P:(m + 1) * P], hs[j][:, :C],
                            start=(j == 0), stop=(j == 3),
                        )
                    if m % 2 == 0:
                        nc.vector.tensor_copy(yo[:, m, :], pd[:, :C])
                    else:
                        nc.scalar.activation(yo[:, m, :], pd[:, :C], f.Copy)
                nc.gpsimd.dma_start(ymoe_d[e].rearrange("p (k t) -> p k t", k=KD), yo[:])

            # ---- shared expert shard ----
            for t_i in range(NT):
                xth = xt[t_i // 2]
                tsl = slice((t_i % 2) * TT, (t_i % 2 + 1) * TT)
                ysh_t = ysh_d[t_i].rearrange("p (k t) -> p k t", k=KD)
                hsh = []
                for j, (off, sz) in enumerate(FF_CH):
                    pg = psum.tile([P, TT], dt.float32, tag="pg", name=f"spg{t_i}_{j}")
                    pu = psum.tile([P, TT], dt.float32, tag="pu", name=f"spu{t_i}_{j}")
                    for k in range(KD):
                        mm = nc.tensor.matmul(
                            pg[:sz, :], wg[:, k, off:off + sz], xth[:, k, tsl],
                            start=(k == 0), stop=(k == KD - 1),
                        )
                        if t_i == 0 and j == 0 and k == 0:
                            stage_gate[2] = mm
                    for k in range(KD):
                        nc.tensor.matmul(
                            pu[:sz, :], wu[:, k, off:off + sz], xth[:, k, tsl],
                            start=(k == 0), stop=(k == KD - 1),
                        )
                    sg = acts.tile([P, TT], dt.bfloat16, tag="sg", name=f"ssg{t_i}_{j}")
                    nc.scalar.activation(sg[:sz, :], pg[:sz, :], f.Silu)
                    h = acts.tile([P, TT], dt.bfloat16, tag=f"h{j}", name=f"sh{t_i}_{j}")
                    nc.vector.tensor_tensor(h[:sz, :], sg[:sz, :], pu[:sz, :], mybir.AluOpType.mult)
                    hsh.append(h)
                ys = outs.tile([P, KD, TT], dt.bfloat16, tag="ysh", name=f"ysh{t_i}")
                for m in range(KD):
                    pd = psum.tile([P, TT], dt.float32, tag="pd", bufs=3, name=f"spd{t_i}_{m}")
                    for j, (off, sz) in enumerate(FF_CH):
                        nc.tensor.matmul(
                            pd[:], wdt[j][:sz, m * P:(m + 1) * P], hsh[j][:sz, :],
                            start=(j == 0), stop=(j == 2),
                        )
                    if m % 2 == 0:
                        nc.vector.tensor_copy(ys[:, m, :], pd[:])
                    else:
                        nc.scalar.activation(ys[:, m, :], pd[:], f.Copy)
                    if m == 3:
                        nc.gpsimd.dma_start(ysh_t[:, 0:4, :], ys[:, 0:4, :])
                nc.gpsimd.dma_start(ysh_t[:, 4:8, :], ys[:, 4:8, :])

            # release each stage's loads only once the previous phase's
            # compute has started, so early transfers get full HBM bandwidth
            for s in (1, 2, 3):
                for d in stage_dmas[s]:
                    add_dep_helper(d.ins, stage_gate[s - 1].ins, sync=True,
                                   reason=f"stage{s} load gated on phase {s - 1}")
    _split_excess_waits(nc)
    return nc


def _route(x: np.ndarray, gate_w: np.ndarray):
    logits = x @ gate_w.T
    logits = logits.astype(np.float32)
    m = logits.max(axis=-1, keepdims=True)
    p = np.exp(logits - m)
    p /= p.sum(axis=-1, keepdims=True)
    sel = np.argsort(-p, axis=-1, kind="stable")[:, :TOP_K]
    rw = np.take_along_axis(p, sel, axis=-1)
    rw = rw / rw.sum(axis=-1, keepdims=True)
    idxs, wts = [], []
    for e in range(NUM_EXPERTS):
        mask = (sel == e).any(axis=-1)
        idx = np.nonzero(mask)[0]
        w = rw[idx][sel[idx] == e]
        idxs.append(idx)
        wts.append(w.astype(np.float32))
    return idxs, wts


def kernel(layer_input, gate_w, w_gate_up, w_down,
           shared_w_gate, shared_w_up, shared_w_down, shared_gate_w):
    B, S, D = layer_input.shape
    x = np.ascontiguousarray(np.asarray(layer_input, dtype=np.float32).reshape(-1, D))

    idxs, wts = _route(x, np.asarray(gate_w, dtype=np.float32))
    cmax = max(len(i) for i in idxs)
    C = max(32, ((cmax + 31) // 32) * 32)

    key = C
    if key not in _prog_cache:
        _prog_cache[key] = _build_program(C)
    nc = _prog_cache[key]

    xtT = x.T  # [1024, 2048]
    xt_pk = np.stack([_pk(np.ascontiguousarray(xtT[:, :NTOK // 2])),
                      _pk(np.ascontiguousarray(xtT[:, NTOK // 2:]))])
    wgu_f = np.asarray(w_gate_up, dtype=np.float32)
    wd_f = np.asarray(w_down, dtype=np.float32)
    wg_t = np.asarray(shared_w_gate, np.float32).T  # [1024, 2816]
    wu_t = np.asarray(shared_w_up, np.float32).T
    wdt_t = np.ascontiguousarray(np.asarray(shared_w_down, np.float32).T).astype(BF16)

    in_maps = []
    for c in range(N_CORES):
        e0, e1 = 2 * c, 2 * c + 1
        xe = np.zeros((2, P, KD * C), dtype=BF16)
        wgu = np.zeros((2, 4, P, KD * 2 * P), dtype=BF16)
        wd = np.zeros((2, P, (MOE_FF // P) * HIDDEN), dtype=BF16)
        for s_i, e in enumerate((e0, e1)):
            cnt = len(idxs[e])
            xfull = np.zeros((C, HIDDEN), dtype=np.float32)
            xfull[:cnt] = x[idxs[e]]
            xe[s_i] = _pk(np.ascontiguousarray(xfull.T))
            for j in range(4):
                blk = np.concatenate(
                    [wgu_f[e][:, j * P:(j + 1) * P],
                     wgu_f[e][:, MOE_FF + j * P:MOE_FF + (j + 1) * P]], axis=1)
                wgu[s_i, j] = _pk(blk)
            wd[s_i] = _pk(wd_f[e])
        fsl = slice(c * FF_SH, (c + 1) * FF_SH)
        in_maps.append({
            "xt": xt_pk,
            "xe": xe,
            "wgu": wgu,
            "wd": wd,
            "wg_t": _pk(np.ascontiguousarray(wg_t[:, fsl])),
            "wu_t": _pk(np.ascontiguousarray(wu_t[:, fsl])),
            "wdt": wdt_t[fsl, :],
        })

    trace = bool(int(os.environ.get("BASS_MOE_TRACE", "0")))
    res = run_bass_kernel_spmd(
        nc, in_maps, core_ids=list(range(N_CORES)),
        trace=trace, trace_cores=list(range(N_CORES)) if trace else None,
    )
    kernel.last_results = res

    # unshard: y_sh [NT, 128, KD*TT] -> [1024, 2048]
    shared = np.zeros((HIDDEN, NTOK), dtype=np.float32)
    for c in range(N_CORES):
        ysh = np.asarray(res.results[c]["y_sh"]).astype(np.float32)
        ysh = ysh.reshape(NT, P, KD, TT).transpose(2, 1, 0, 3).reshape(HIDDEN, NTOK)
        shared += ysh
    sig = 1.0 / (1.0 + np.exp(-(x @ np.asarray(shared_gate_w, np.float32).T)))
    out = shared.T * sig
    for e in range(NUM_EXPERTS):
        c, s_i = e // 2, e % 2
        cnt = len(idxs[e])
        if cnt == 0:
            continue
        ym = np.asarray(res.results[c]["y_moe"][s_i]).astype(np.float32)
        ym = ym.reshape(P, KD, C).transpose(1, 0, 2).reshape(HIDDEN, C)
        out[idxs[e]] += wts[e][:, None] * ym[:, :cnt].T
    return out.reshape(B, S, D).astype(np.float32)
